# revision 1
# baseline (speedup 1.0000x reference)
"""Grouped SwiGLU experts (MoE post-dispatch compute) on 8 Trainium2 cores.

Expert-parallel: host gathers tokens per expert (the "all-to-all dispatch"),
packs 128-token blocks into a uniform per-core segment schedule (specialized
to the actual counts at compile time), each segment runs one expert's SwiGLU
  hT = silu(w1.T x.T) * (w3.T x.T);  out = (hT.T) @ w2
on one NeuronCore in bf16 with fp32 PSUM accumulation, and the host
scatters rows back to their original token positions.
"""

import numpy as np
import ml_dtypes

# ---- problem constants (from the reference module) ----
T, D, H, E, R, ALIGN = 8192, 4096, 1024, 8, 2, 16
P = 128          # partition width
DT = D // P      # 32 d-tiles
HT = H // P      # 8 h-chunks
NCORES = 8
UNIT = 16        # scheduling granularity in rows (reference ALIGN)
MAX_UNITS = 32   # max units per segment = 512 rows (psum bank / matmul N cap)

BF16 = ml_dtypes.bfloat16


def _permute_indices(counts):
    """numpy port of reference._permute_indices."""
    counts = counts.astype(np.int64)
    max_len = T + E * ALIGN
    start_index = np.cumsum(counts) - counts
    total = counts.reshape(R, E).sum(0)
    m_sizes = ((np.maximum(total, ALIGN) + ALIGN - 1) // ALIGN * ALIGN).astype(np.int64)
    m_offsets = np.cumsum(m_sizes)
    write_offsets = m_offsets - m_sizes
    c_er = counts.reshape(R, E).T
    seg_ws = (write_offsets[:, None] + np.cumsum(c_er, 1) - c_er).reshape(-1)
    seg_len = c_er.reshape(-1)
    seg_src = start_index.reshape(R, E).T.reshape(-1)
    pos = np.arange(max_len, dtype=np.int64)
    idx = np.clip(np.searchsorted(seg_ws, pos, side="right") - 1, 0, E * R - 1)
    within = pos - seg_ws[idx]
    valid = (within >= 0) & (within < seg_len[idx])
    perm = np.where(valid, seg_src[idx] + within, T)
    return perm.astype(np.int64), m_sizes, (m_offsets - m_sizes)


def _partitions(c, max_part, max_len):
    """Partitions of c into <= max_len parts each <= max_part, desc order."""
    out = []

    def rec(rem, mx, cur):
        if rem == 0:
            out.append(tuple(cur))
            return
        if len(cur) == max_len:
            return
        for p in range(min(rem, mx), 0, -1):
            cur.append(p)
            rec(rem - p, p, cur)
            cur.pop()

    rec(c, max_part, [])
    return out


def _ffd(nblk, parts):
    """Pack per-expert block counts into 8 cores x len(parts) bins
    (bin (c,j) capacity parts[j], single expert per bin). Returns
    assignment dict (core, j) -> (expert, block_start, nb) or None."""
    bins = []  # (cap, core, j)
    for c in range(NCORES):
        for j, cap in enumerate(parts):
            bins.append([cap, c, j])
    bins.sort(key=lambda b: -b[0])
    used = [False] * len(bins)
    asg = {}
    order = sorted(range(E), key=lambda e: -nblk[e])
    for e in order:
        rem = int(nblk[e])
        b0 = 0
        while rem > 0:
            # largest unused bin with cap <= rem (fill fully); else the
            # smallest unused bin (minimize slack)
            best_le, best_gt = None, None
            for i, (cap, c, j) in enumerate(bins):
                if used[i]:
                    continue
                if cap <= rem:
                    best_le = i  # bins desc: first such is largest
                    break
                best_gt = i  # keeps updating: last seen = smallest so far
            i = best_le if best_le is not None else best_gt
            if i is None:
                return None
            cap, c, j = bins[i]
            used[i] = True
            nb = min(cap, rem)
            asg[(c, j)] = (e, b0, nb)
            b0 += nb
            rem -= nb
    return asg


def _plan(nblk):
    """nblk: per-expert sizes in UNIT-row units. Returns (parts, asg) with
    parts = per-segment capacities in units (same on all cores)."""
    n = int(sum(nblk))
    c0 = max(1, -(-n // NCORES))
    # a segment costs a full weight stream (~25MB) regardless of size, so
    # prefer <=2 segments even at a few extra rows before allowing 3
    for max_len, c_hi in ((2, c0 + 8), (3, c0 + 8), (3, 600)):
        for c in range(c0, c_hi):
            opts = _partitions(c, MAX_UNITS, max_len)
            # most balanced first (largest min part)
            for parts in sorted(opts, key=lambda p: (len(p), -min(p))):
                asg = _ffd(nblk, parts)
                if asg is not None:
                    return list(parts), asg
    raise RuntimeError("packing failed")


def _build_program(parts):
    import concourse.mybir as mybir
    import concourse.tile as tile
    from concourse import bacc

    bf = mybir.dt.bfloat16
    f32 = mybir.dt.float32
    SILU = mybir.ActivationFunctionType.Silu

    nc = bacc.Bacc("TRN2", target_bir_lowering=False, debug=False,
                   num_devices=NCORES)

    xt_d, w1_d, w3_d, w2_d, out_d = [], [], [], [], []
    for j, U in enumerate(parts):
        M = U * UNIT
        xt_d.append(nc.dram_tensor(f"xt{j}", [DT, P, M], bf, kind="ExternalInput"))
        w1_d.append(nc.dram_tensor(f"w1p{j}", [HT, P, D], bf, kind="ExternalInput"))
        w3_d.append(nc.dram_tensor(f"w3p{j}", [HT, P, D], bf, kind="ExternalInput"))
        w2_d.append(nc.dram_tensor(f"w2p{j}", [H, D], bf, kind="ExternalInput"))
        out_d.append(nc.dram_tensor(f"out{j}", [M, D], bf, kind="ExternalOutput"))

    with tile.TileContext(nc) as tc:
        with (
            tc.tile_pool(name="xt", bufs=2 * DT) as xt_pool,
            tc.tile_pool(name="wp", bufs=10) as wp_pool,
            tc.tile_pool(name="w2", bufs=10) as w2_pool,
            tc.tile_pool(name="ht", bufs=2 * HT) as ht_pool,
            tc.tile_pool(name="stmp", bufs=2) as stmp_pool,
            tc.tile_pool(name="ost", bufs=6) as ost_pool,
            tc.tile_pool(name="ps1", bufs=2, space="PSUM") as ps1_pool,
            tc.tile_pool(name="ps3", bufs=2, space="PSUM") as ps3_pool,
            tc.tile_pool(name="pso", bufs=3, space="PSUM") as pso_pool,
            tc.tile_pool(name="warm", bufs=1) as warm_pool,
            tc.tile_pool(name="pwarm", bufs=1, space="PSUM") as pwarm_pool,
        ):
            # keep the PE busy (HAM at K=8/8) while the first real DMAs land
            wz = warm_pool.tile([P, P], bf, tag="warm", name="warmz")
            nc.gpsimd.memset(wz[:], 0.0)
            pw = pwarm_pool.tile([P, P], f32, tag="pwarm", name="warmp")
            for _ in range(140):
                nc.tensor.matmul(pw[:], wz[:], wz[:], start=True, stop=True)
            for j, U in enumerate(parts):
                M = U * UNIT

                def load_wp(h, j=j):
                    # half-panels: first matmuls only wait on 512KB, and
                    # panel loads pipeline at finer grain
                    DH = D // 2
                    tiles = []
                    for half in range(2):
                        t1 = wp_pool.tile([P, DH], bf, tag="wp",
                                          name=f"w1p{j}_{h}_{half}")
                        nc.sync.dma_start(
                            out=t1[:], in_=w1_d[j][h, :, half * DH:(half + 1) * DH])
                        t3 = wp_pool.tile([P, DH], bf, tag="wp",
                                          name=f"w3p{j}_{h}_{half}")
                        nc.sync.dma_start(
                            out=t3[:], in_=w3_d[j][h, :, half * DH:(half + 1) * DH])
                        tiles.append((t1, t3))
                    return tiles

                # critical path first: h=0 weight panels, then token tiles;
                # w2 (phase 2 only) is deferred until after phase 1 emission
                wp_cur = load_wp(0)
                xts = []
                for d in range(DT):
                    t = xt_pool.tile([P, M], bf, tag="xt", name=f"xt{j}_{d}")
                    nc.sync.dma_start(out=t[:], in_=xt_d[j][d])
                    xts.append(t)
                hts = [ht_pool.tile([P, M], bf, tag="ht", name=f"ht{j}_{h}")
                       for h in range(HT)]
                for h in range(HT):
                    wp_halves = wp_cur
                    if h + 1 < HT:
                        wp_cur = load_wp(h + 1)
                    ps1 = ps1_pool.tile([P, M], f32, tag="ps1")
                    ps3 = ps3_pool.tile([P, M], f32, tag="ps3")
                    DH = DT // 2
                    for d in range(DT):
                        w1p, w3p = wp_halves[d // DH]
                        dd = d % DH
                        nc.tensor.matmul(ps1[:], w1p[:, dd * P:(dd + 1) * P],
                                         xts[d][:], start=(d == 0),
                                         stop=(d == DT - 1))
                        nc.tensor.matmul(ps3[:], w3p[:, dd * P:(dd + 1) * P],
                                         xts[d][:], start=(d == 0),
                                         stop=(d == DT - 1))
                    tmp = stmp_pool.tile([P, M], f32, tag="stmp")
                    nc.scalar.activation(tmp[:], ps1[:], SILU)
                    nc.vector.tensor_mul(hts[h][:], tmp[:], ps3[:])
                w2s = []
                for h in range(HT):
                    t = w2_pool.tile([P, D], bf, tag="w2", name=f"w2{j}_{h}")
                    nc.sync.dma_start(out=t[:], in_=w2_d[j][h * P:(h + 1) * P, :])
                    w2s.append(t)
                NB = (M + P - 1) // P
                for b in range(NB):
                    pb = min(P, M - b * P)  # last block may be partial
                    for dc in range(D // 512):
                        po = pso_pool.tile([P, 512], f32, tag="pso")
                        for h in range(HT):
                            nc.tensor.matmul(
                                po[:pb, :], hts[h][:, b * P:b * P + pb],
                                w2s[h][:, dc * 512:(dc + 1) * 512],
                                start=(h == 0), stop=(h == HT - 1))
                        ob = ost_pool.tile([P, 512], bf, tag="ost")
                        nc.vector.tensor_copy(ob[:pb, :], po[:pb, :])
                        nc.sync.dma_start(
                            out=out_d[j][b * P:b * P + pb, dc * 512:(dc + 1) * 512],
                            in_=ob[:pb, :])

    nc.compile()
    return nc


_CACHE = {}


def _get_program(parts):
    key = tuple(parts)
    if key not in _CACHE:
        _CACHE[key] = _build_program(parts)
    return _CACHE[key]


_LAST_RESULT = None


def kernel(x, w1, w2, w3, num_tokens_per_expert):
    import os
    from concourse.bass_utils import run_bass_kernel_spmd

    x = np.asarray(x, dtype=np.float32)
    counts = np.asarray(num_tokens_per_expert).astype(np.int64)
    perm, m_sizes, m_off = _permute_indices(counts)
    nunits = m_sizes // UNIT  # m_sizes are UNIT-aligned

    parts, asg = _plan(nunits)
    nc = _get_program(parts)

    # expert-grouped token stream (the dispatch): rows of x per expert
    x_pad = np.concatenate([x, np.zeros((1, D), np.float32)], axis=0)
    ltot = int(m_sizes.sum())
    xp = x_pad[perm[:ltot]]  # [ltot, D] expert-grouped, 16-aligned per expert
    xe = [xp[m_off[e]:m_off[e] + m_sizes[e]] for e in range(E)]

    w1b = [np.ascontiguousarray(
        np.asarray(w1[e], np.float32).reshape(DT, P, HT, P)
        .transpose(2, 1, 0, 3).reshape(HT, P, D)).astype(BF16) for e in range(E)]
    w3b = [np.ascontiguousarray(
        np.asarray(w3[e], np.float32).reshape(DT, P, HT, P)
        .transpose(2, 1, 0, 3).reshape(HT, P, D)).astype(BF16) for e in range(E)]
    w2b = [np.asarray(w2[e], np.float32).astype(BF16) for e in range(E)]

    in_maps = []
    for c in range(NCORES):
        m = {}
        for j, U in enumerate(parts):
            M = U * UNIT
            ent = asg.get((c, j))
            e = ent[0] if ent is not None else 0
            blk = np.zeros((M, D), np.float32)
            if ent is not None:
                _, u0, nu = ent
                blk[:nu * UNIT] = xe[e][u0 * UNIT:(u0 + nu) * UNIT]
            # [DT, P, M]: xt[d, p, m] = blk[m, d*P+p]
            m[f"xt{j}"] = np.ascontiguousarray(
                blk.reshape(M, DT, P).transpose(1, 2, 0)).astype(BF16)
            m[f"w1p{j}"] = w1b[e]
            m[f"w3p{j}"] = w3b[e]
            m[f"w2p{j}"] = w2b[e]
        in_maps.append(m)

    kw = {}
    if os.environ.get("KERNEL_TRACE"):
        kw = dict(trace=True, tmpdir=os.environ.get("KERNEL_TRACE_DIR") or None)
    res = run_bass_kernel_spmd(nc, in_maps, core_ids=list(range(NCORES)), **kw)
    global _LAST_RESULT
    _LAST_RESULT = res

    # reassemble expert-grouped output stream, then scatter to token order
    outp = np.zeros((ltot, D), np.float32)
    for (c, j), (e, u0, nu) in asg.items():
        nr = nu * UNIT
        outp[m_off[e] + u0 * UNIT:m_off[e] + u0 * UNIT + nr] = \
            res.results[c][f"out{j}"][:nr]

    out = np.zeros((T + 1, D), np.float32)
    out[perm[:ltot]] = outp
    return out[:T]



# revision 2
# speedup vs baseline: 1.0480x; 1.0480x over previous
"""Grouped SwiGLU experts (MoE post-dispatch compute) on 8 Trainium2 cores.

Expert-parallel with a compile-time segment schedule (one expert per
segment, uniform capacities across cores so a single SPMD program serves
all 8 cores). Per segment:

  big  (M>128 rows):  phase1  hT[h,M]  = silu(w1.T x.T) * (w3.T x.T)
                      phase2  outT[d,M] = w2[d-slice].T @ hT   (w2 stationary,
                      N=M moving)  -- no partial-block waste.
  small (M<=128 rows): operands swapped so the moving operand is the
                      weight panel at N=512 (tokens are the stationary
                      128-col operand), h' is DMA-transposed between the
                      phases, and all three weight matrices stream as
                      float8-e3m4 (pow2-scaled) to halve their HBM traffic
                      -- small segments are weight-DMA bound otherwise.

DMA queues: sync=weights, scalar=outputs+transposes, gpsimd=tokens, every
transfer >=0.8MB with >=6.6KB lines. Outputs return in bf16; the host
packs/gathers/scatters (not part of HW exec time).
"""

import numpy as np
import ml_dtypes

# ---- problem constants (from the reference module) ----
T, D, H, E, R, ALIGN = 8192, 4096, 1024, 8, 2, 16
P = 128          # partition width
DT = D // P      # 32 d-tiles
HT = H // P      # 8 h-chunks
NCORES = 8
UNIT = 16        # scheduling granularity in rows (reference ALIGN)
MAX_UNITS = 32   # max units per segment = 512 rows (psum bank / matmul N cap)
SMALL_ROWS = 128  # segments at or below this use the fp8 swapped path

BF16 = ml_dtypes.bfloat16
F8E3 = ml_dtypes.float8_e3m4
S1 = 128.0       # fp8 scale for w1/w3 (weights ~N(0, 1/64): 128*5sigma ~ 10)
S2 = 8192.0      # psum scale of fp8 phase-2 output; w2 scale = S2/S1 = 64


def _permute_indices(counts):
    """numpy port of reference._permute_indices."""
    counts = counts.astype(np.int64)
    max_len = T + E * ALIGN
    start_index = np.cumsum(counts) - counts
    total = counts.reshape(R, E).sum(0)
    m_sizes = ((np.maximum(total, ALIGN) + ALIGN - 1) // ALIGN * ALIGN).astype(np.int64)
    m_offsets = np.cumsum(m_sizes)
    write_offsets = m_offsets - m_sizes
    c_er = counts.reshape(R, E).T
    seg_ws = (write_offsets[:, None] + np.cumsum(c_er, 1) - c_er).reshape(-1)
    seg_len = c_er.reshape(-1)
    seg_src = start_index.reshape(R, E).T.reshape(-1)
    pos = np.arange(max_len, dtype=np.int64)
    idx = np.clip(np.searchsorted(seg_ws, pos, side="right") - 1, 0, E * R - 1)
    within = pos - seg_ws[idx]
    valid = (within >= 0) & (within < seg_len[idx])
    perm = np.where(valid, seg_src[idx] + within, T)
    return perm.astype(np.int64), m_sizes, (m_offsets - m_sizes)


def _partitions(c, max_part, max_len):
    """Partitions of c into <= max_len parts each <= max_part, desc order."""
    out = []

    def rec(rem, mx, cur):
        if rem == 0:
            out.append(tuple(cur))
            return
        if len(cur) == max_len:
            return
        for p in range(min(rem, mx), 0, -1):
            cur.append(p)
            rec(rem - p, p, cur)
            cur.pop()

    rec(c, max_part, [])
    return out


def _ffd(nblk, parts):
    """Pack per-expert block counts into 8 cores x len(parts) bins
    (bin (c,j) capacity parts[j], single expert per bin). Returns
    assignment dict (core, j) -> (expert, block_start, nb) or None."""
    bins = []  # (cap, core, j)
    for c in range(NCORES):
        for j, cap in enumerate(parts):
            bins.append([cap, c, j])
    bins.sort(key=lambda b: -b[0])
    used = [False] * len(bins)
    asg = {}
    order = sorted(range(E), key=lambda e: -nblk[e])
    for e in order:
        rem = int(nblk[e])
        b0 = 0
        while rem > 0:
            # largest unused bin with cap <= rem (fill fully); else the
            # smallest unused bin (minimize slack)
            best_le, best_gt = None, None
            for i, (cap, c, j) in enumerate(bins):
                if used[i]:
                    continue
                if cap <= rem:
                    best_le = i  # bins desc: first such is largest
                    break
                best_gt = i  # keeps updating: last seen = smallest so far
            i = best_le if best_le is not None else best_gt
            if i is None:
                return None
            cap, c, j = bins[i]
            used[i] = True
            nb = min(cap, rem)
            asg[(c, j)] = (e, b0, nb)
            b0 += nb
            rem -= nb
    return asg


def _plan(nblk):
    """nblk: per-expert sizes in UNIT-row units. Returns (parts, asg) with
    parts = per-segment capacities in units (same on all cores)."""
    n = int(sum(nblk))
    c0 = max(1, -(-n // NCORES))
    # a segment costs a full weight stream regardless of size, so prefer
    # <=2 segments even at a few extra rows before allowing 3
    for max_len, c_hi in ((2, c0 + 8), (3, c0 + 8), (3, 600)):
        for c in range(c0, c_hi):
            opts = _partitions(c, MAX_UNITS, max_len)
            # most balanced first (largest min part)
            for parts in sorted(opts, key=lambda p: (len(p), -min(p))):
                asg = _ffd(nblk, parts)
                if asg is not None:
                    return list(parts), asg
    raise RuntimeError("packing failed")


def _seg_kinds(parts):
    return ["small" if U * UNIT <= SMALL_ROWS else "big" for U in parts]


def _build_program(parts):
    import concourse.mybir as mybir
    import concourse.tile as tile
    from concourse import bacc

    bf = mybir.dt.bfloat16
    f8 = mybir.dt.float8e3
    f32 = mybir.dt.float32
    SILU = mybir.ActivationFunctionType.Silu

    nc = bacc.Bacc("TRN2", target_bir_lowering=False, debug=False,
                   num_devices=NCORES)

    kinds = _seg_kinds(parts)
    dram = []
    for j, U in enumerate(parts):
        M = U * UNIT
        t = {}
        t["xt"] = nc.dram_tensor(f"xt{j}", [P, DT * M], bf, kind="ExternalInput")
        if kinds[j] == "big":
            t["w1"] = nc.dram_tensor(f"w1p{j}", [HT, P, D], bf, kind="ExternalInput")
            t["w3"] = nc.dram_tensor(f"w3p{j}", [HT, P, D], bf, kind="ExternalInput")
            t["w2"] = nc.dram_tensor(f"w2p{j}", [H, D], bf, kind="ExternalInput")
            t["out"] = nc.dram_tensor(f"out{j}", [P, DT * M], bf, kind="ExternalOutput")
        else:
            t["w1"] = nc.dram_tensor(f"w1p{j}", [P, DT * H], f8, kind="ExternalInput")
            t["w3"] = nc.dram_tensor(f"w3p{j}", [P, DT * H], f8, kind="ExternalInput")
            t["w2"] = nc.dram_tensor(f"w2p{j}", [P, HT * D], f8, kind="ExternalInput")
            t["out"] = nc.dram_tensor(f"out{j}", [M, D], bf, kind="ExternalOutput")
        dram.append(t)

    with tile.TileContext(nc) as tc:
        with (
            tc.tile_pool(name="xtb", bufs=4) as xtb_pool,    # big token chunks
            tc.tile_pool(name="xts", bufs=2) as xts_pool,    # small token tile
            tc.tile_pool(name="w8", bufs=5) as w8_pool,      # w1/w3 streams (8KB slots)
            tc.tile_pool(name="w2", bufs=10) as w2_pool,     # w2 streams (8KB slots)
            tc.tile_pool(name="ht", bufs=8) as ht_pool,      # big h tiles
            tc.tile_pool(name="htp", bufs=1) as htp_pool,    # small h' [M, H]
            tc.tile_pool(name="htt", bufs=1) as htt_pool,    # small h transposed
            tc.tile_pool(name="tmp", bufs=2) as tmp_pool,    # silu temp
            tc.tile_pool(name="ob", bufs=2) as ob_pool,      # output staging
            tc.tile_pool(name="warm", bufs=1) as warm_pool,
            tc.tile_pool(name="pacc", bufs=4, space="PSUM") as pacc_pool,
            tc.tile_pool(name="pout", bufs=3, space="PSUM") as pout_pool,
            tc.tile_pool(name="pwarm", bufs=1, space="PSUM") as pwarm_pool,
        ):
            # keep the PE busy while the first weight/token DMAs land
            wz = warm_pool.tile([P, P], bf, tag="warm", name="warmz")
            nc.vector.memset(wz[:], 0.0)
            pw = pwarm_pool.tile([P, P], f32, tag="pwarm", name="warmp")
            for _ in range(48):
                nc.tensor.matmul(pw[:], wz[:], wz[:], start=True, stop=True)

            for j, U in enumerate(parts):
                M = U * UNIT
                t = dram[j]
                if kinds[j] == "big":
                    # ---- tokens: 4 chunks of 8 d-tiles, SWDGE queue ----
                    xch = []
                    for c in range(4):
                        xt = xtb_pool.tile([P, 8 * M], bf, tag="xtb",
                                           name=f"xt{j}_{c}")
                        nc.gpsimd.dma_start(
                            out=xt[:], in_=t["xt"][:, c * 8 * M:(c + 1) * 8 * M])
                        xch.append(xt)

                    def load_wp(h, j=j, t=t):
                        w1t = w8_pool.tile([P, D], bf, tag="w8", name=f"w1_{j}_{h}")
                        nc.sync.dma_start(out=w1t[:], in_=t["w1"][h])
                        w3t = w8_pool.tile([P, D], bf, tag="w8", name=f"w3_{j}_{h}")
                        nc.sync.dma_start(out=w3t[:], in_=t["w3"][h])
                        return w1t, w3t

                    wp_cur = load_wp(0)
                    hts, w2s = [], []
                    for h in range(HT):
                        w1t, w3t = wp_cur
                        if h + 1 < HT:
                            wp_cur = load_wp(h + 1)
                        # w2 panels stream through phase 1 on the sync queue
                        w2t = w2_pool.tile([P, D], bf, tag="w2", name=f"w2_{j}_{h}")
                        nc.sync.dma_start(out=w2t[:], in_=t["w2"][h * P:(h + 1) * P, :])
                        w2s.append(w2t)
                        ps1 = pacc_pool.tile([P, M], f32, tag="pacc", name=f"ps1_{j}_{h}")
                        ps3 = pacc_pool.tile([P, M], f32, tag="pacc", name=f"ps3_{j}_{h}")
                        for d in range(DT):
                            rhs = xch[d // 8][:, (d % 8) * M:(d % 8 + 1) * M]
                            nc.tensor.matmul(ps1[:], w1t[:, d * P:(d + 1) * P], rhs,
                                             start=(d == 0), stop=(d == DT - 1))
                            nc.tensor.matmul(ps3[:], w3t[:, d * P:(d + 1) * P], rhs,
                                             start=(d == 0), stop=(d == DT - 1))
                        tmp = tmp_pool.tile([P, max(M, 512)], f32, tag="tmp",
                                            name=f"tmp_{j}_{h}")
                        nc.scalar.activation(tmp[:, :M], ps1[:], SILU)
                        ht = ht_pool.tile([P, M], bf, tag="ht", name=f"ht_{j}_{h}")
                        nc.vector.tensor_mul(ht[:], tmp[:, :M], ps3[:])
                        hts.append(ht)
                    # ---- phase 2 (w2 stationary): outT[dt] = sum_h w2[h,dt].T @ ht[h]
                    ob = None
                    for dt in range(DT):
                        po = pout_pool.tile([P, max(M, 512)], f32, tag="pout",
                                            name=f"po_{j}_{dt}")
                        for h in range(HT):
                            nc.tensor.matmul(po[:, :M], w2s[h][:, dt * P:(dt + 1) * P],
                                             hts[h][:], start=(h == 0),
                                             stop=(h == HT - 1))
                        if dt % 8 == 0:
                            ob = ob_pool.tile([P, max(8 * M, D)], bf, tag="ob",
                                              name=f"ob_{j}_{dt // 8}")
                        nc.vector.tensor_copy(ob[:, (dt % 8) * M:(dt % 8 + 1) * M],
                                              po[:, :M])
                        if dt % 8 == 7:
                            c = dt // 8
                            nc.scalar.dma_start(
                                out=t["out"][:, c * 8 * M:(c + 1) * 8 * M],
                                in_=ob[:, :8 * M])
                else:
                    # ---- small segment: tokens stationary, fp8 weights moving
                    xt = xts_pool.tile([P, DT * M], bf, tag="xts", name=f"xts{j}")
                    nc.gpsimd.dma_start(out=xt[:], in_=t["xt"][:])
                    w13 = []
                    for c in range(4):
                        w1t = w8_pool.tile([P, 8 * H], f8, tag="w8", name=f"w1s_{j}_{c}")
                        nc.sync.dma_start(out=w1t[:],
                                          in_=t["w1"][:, c * 8 * H:(c + 1) * 8 * H])
                        w3t = w8_pool.tile([P, 8 * H], f8, tag="w8", name=f"w3s_{j}_{c}")
                        nc.sync.dma_start(out=w3t[:],
                                          in_=t["w3"][:, c * 8 * H:(c + 1) * 8 * H])
                        w13.append((w1t, w3t))
                    w2c = []
                    for c in range(4):
                        w2t = w2_pool.tile([P, 2 * D], f8, tag="w2", name=f"w2s_{j}_{c}")
                        nc.sync.dma_start(out=w2t[:],
                                          in_=t["w2"][:, c * 2 * D:(c + 1) * 2 * D])
                        w2c.append(w2t)
                    pa = [pacc_pool.tile([P, 512], f32, tag="pacc", name=f"pa_{j}_{i}")
                          for i in range(2)]
                    pb = [pacc_pool.tile([P, 512], f32, tag="pacc", name=f"pb_{j}_{i}")
                          for i in range(2)]
                    for d in range(DT):
                        lhsT = xt[:, d * M:(d + 1) * M]
                        w1t, w3t = w13[d // 8]
                        off = (d % 8) * H
                        st, sp = (d == 0), (d == DT - 1)
                        nc.tensor.matmul(pa[0][:M, :], lhsT, w1t[:, off:off + 512],
                                         start=st, stop=sp)
                        nc.tensor.matmul(pa[1][:M, :], lhsT, w1t[:, off + 512:off + 1024],
                                         start=st, stop=sp)
                        nc.tensor.matmul(pb[0][:M, :], lhsT, w3t[:, off:off + 512],
                                         start=st, stop=sp)
                        nc.tensor.matmul(pb[1][:M, :], lhsT, w3t[:, off + 512:off + 1024],
                                         start=st, stop=sp)
                    htp = htp_pool.tile([P, H], bf, tag="htp", name=f"htp{j}")
                    for i in range(2):
                        tmp = tmp_pool.tile([P, max(512, 512)], f32, tag="tmp",
                                            name=f"tmps_{j}_{i}")
                        nc.scalar.activation(tmp[:M, :512], pa[i][:M, :], SILU,
                                             scale=1.0 / S1)
                        nc.vector.tensor_mul(htp[:M, i * 512:(i + 1) * 512],
                                             tmp[:M, :512], pb[i][:M, :])
                    # h' [M, H] -> hT [h, M] via xbar transpose (scalar HWDGE)
                    htt = htt_pool.tile([P, HT * M], bf, tag="htt", name=f"htt{j}")
                    for h in range(HT):
                        nc.scalar.dma_start(out=htt[:, h * M:(h + 1) * M],
                                            in_=htp[:M, h * P:(h + 1) * P],
                                            transpose=True)
                    obs = ob_pool.tile([P, max(8 * M, D)], bf, tag="ob", name=f"obs{j}")
                    for dc in range(D // 512):
                        po = pout_pool.tile([P, 512], f32, tag="pout", name=f"pos_{j}_{dc}")
                        for h in range(HT):
                            w2t = w2c[h // 2]
                            off = (h % 2) * D + dc * 512
                            nc.tensor.matmul(po[:M, :], htt[:, h * M:(h + 1) * M],
                                             w2t[:, off:off + 512],
                                             start=(h == 0), stop=(h == HT - 1))
                        nc.vector.tensor_scalar_mul(obs[:M, dc * 512:(dc + 1) * 512],
                                                    po[:M, :], 1.0 / S2)
                    nc.scalar.dma_start(out=t["out"][:, :], in_=obs[:M, :D])

    nc.compile()
    return nc


_CACHE = {}


def _get_program(parts):
    key = tuple(parts)
    if key not in _CACHE:
        _CACHE[key] = _build_program(parts)
    return _CACHE[key]


_LAST_RESULT = None


def kernel(x, w1, w2, w3, num_tokens_per_expert):
    import os
    from concourse.bass_utils import run_bass_kernel_spmd

    x = np.asarray(x, dtype=np.float32)
    w1 = np.asarray(w1, np.float32)
    w2 = np.asarray(w2, np.float32)
    w3 = np.asarray(w3, np.float32)
    counts = np.asarray(num_tokens_per_expert).astype(np.int64)
    perm, m_sizes, m_off = _permute_indices(counts)
    nunits = m_sizes // UNIT  # m_sizes are UNIT-aligned

    parts, asg = _plan(nunits)
    kinds = _seg_kinds(parts)
    nc = _get_program(parts)

    # expert-grouped token stream (the dispatch): rows of x per expert
    x_pad = np.concatenate([x, np.zeros((1, D), np.float32)], axis=0)
    ltot = int(m_sizes.sum())
    xp = x_pad[perm[:ltot]]  # [ltot, D] expert-grouped, 16-aligned per expert
    xe = [xp[m_off[e]:m_off[e] + m_sizes[e]] for e in range(E)]

    big_cache, small_cache = {}, {}

    def big_w(e):
        if e not in big_cache:
            w1b = np.ascontiguousarray(
                w1[e].reshape(DT, P, HT, P).transpose(2, 1, 0, 3)
                .reshape(HT, P, D)).astype(BF16)
            w3b = np.ascontiguousarray(
                w3[e].reshape(DT, P, HT, P).transpose(2, 1, 0, 3)
                .reshape(HT, P, D)).astype(BF16)
            w2b = w2[e].astype(BF16)
            big_cache[e] = (w1b, w3b, w2b)
        return big_cache[e]

    def small_w(e):
        if e not in small_cache:
            q1 = np.ascontiguousarray(
                np.clip(w1[e] * S1, -15.0, 15.0).reshape(DT, P, H)
                .transpose(1, 0, 2).reshape(P, DT * H)).astype(F8E3)
            q3 = np.ascontiguousarray(
                np.clip(w3[e] * S1, -15.0, 15.0).reshape(DT, P, H)
                .transpose(1, 0, 2).reshape(P, DT * H)).astype(F8E3)
            q2 = np.ascontiguousarray(
                np.clip(w2[e] * (S2 / S1), -15.0, 15.0).reshape(HT, P, D)
                .transpose(1, 0, 2).reshape(P, HT * D)).astype(F8E3)
            small_cache[e] = (q1, q3, q2)
        return small_cache[e]

    in_maps = []
    for c in range(NCORES):
        m = {}
        for j, U in enumerate(parts):
            M = U * UNIT
            ent = asg.get((c, j))
            e = ent[0] if ent is not None else 0
            blk = np.zeros((M, D), np.float32)
            if ent is not None:
                _, u0, nu = ent
                blk[:nu * UNIT] = xe[e][u0 * UNIT:(u0 + nu) * UNIT]
            # [P, DT*M]: xt[p, d*M + m] = blk[m, d*P + p]
            m[f"xt{j}"] = np.ascontiguousarray(
                blk.reshape(M, DT, P).transpose(2, 1, 0).reshape(P, DT * M)
            ).astype(BF16)
            w1b, w3b, w2b = big_w(e) if kinds[j] == "big" else small_w(e)
            m[f"w1p{j}"] = w1b
            m[f"w3p{j}"] = w3b
            m[f"w2p{j}"] = w2b
        in_maps.append(m)

    kw = {}
    if os.environ.get("KERNEL_TRACE"):
        kw = dict(trace=True, tmpdir=os.environ.get("KERNEL_TRACE_DIR") or None)
    res = run_bass_kernel_spmd(nc, in_maps, core_ids=list(range(NCORES)), **kw)
    global _LAST_RESULT
    _LAST_RESULT = res

    # reassemble expert-grouped output stream, then scatter to token order
    outp = np.zeros((ltot, D), np.float32)
    for (c, j), (e, u0, nu) in asg.items():
        nr = nu * UNIT
        M = parts[j] * UNIT
        r = np.asarray(res.results[c][f"out{j}"], dtype=np.float32)
        if kinds[j] == "big":
            # [P, DT*M] -> [M, D]
            r = r.reshape(P, DT, M).transpose(2, 1, 0).reshape(M, D)
        outp[m_off[e] + u0 * UNIT:m_off[e] + u0 * UNIT + nr] = r[:nr]

    out = np.zeros((T + 1, D), np.float32)
    out[perm[:ltot]] = outp
    return out[:T]


# revision 9
# speedup vs baseline: 1.1571x; 1.1041x over previous
"""Grouped SwiGLU experts (MoE post-dispatch compute) on 8 Trainium2 cores.

Expert-parallel with a compile-time segment schedule (one expert per
segment, uniform capacities across cores so a single SPMD program serves
all 8 cores). Per segment:

  big  (M>128 rows):  phase1  hT[h,M]  = silu(w1.T x.T) * (w3.T x.T)
                      phase2  outT[d,M] = w2[d-slice].T @ hT   (w2 stationary,
                      N=M moving)  -- no partial-block waste.
  small (M<=128 rows): operands swapped so the moving operand is the
                      weight panel at N=512 (tokens are the stationary
                      128-col operand), h' is DMA-transposed between the
                      phases, and all three weight matrices stream as
                      float8-e3m4 (pow2-scaled) to halve their HBM traffic
                      -- small segments are weight-DMA bound otherwise.

DMA queues: sync=weights, scalar=outputs+transposes, gpsimd=tokens, every
transfer >=0.8MB with >=6.6KB lines. Outputs return in bf16; the host
packs/gathers/scatters (not part of HW exec time).
"""

import numpy as np
import ml_dtypes

# ---- problem constants (from the reference module) ----
T, D, H, E, R, ALIGN = 8192, 4096, 1024, 8, 2, 16
P = 128          # partition width
DT = D // P      # 32 d-tiles
HT = H // P      # 8 h-chunks
NCORES = 8
UNIT = 16        # scheduling granularity in rows (reference ALIGN)
MAX_UNITS = 32   # max units per segment = 512 rows (psum bank / matmul N cap)
SMALL_ROWS = 128  # segments at or below this use the fp8 swapped path

BF16 = ml_dtypes.bfloat16
F8E3 = ml_dtypes.float8_e3m4
S1 = 128.0       # fp8 scale for w1/w3 (weights ~N(0, 1/64): 128*5sigma ~ 10)
S2 = 8192.0      # psum scale of fp8 phase-2 output; w2 scale = S2/S1 = 64


def _permute_indices(counts):
    """numpy port of reference._permute_indices."""
    counts = counts.astype(np.int64)
    max_len = T + E * ALIGN
    start_index = np.cumsum(counts) - counts
    total = counts.reshape(R, E).sum(0)
    m_sizes = ((np.maximum(total, ALIGN) + ALIGN - 1) // ALIGN * ALIGN).astype(np.int64)
    m_offsets = np.cumsum(m_sizes)
    write_offsets = m_offsets - m_sizes
    c_er = counts.reshape(R, E).T
    seg_ws = (write_offsets[:, None] + np.cumsum(c_er, 1) - c_er).reshape(-1)
    seg_len = c_er.reshape(-1)
    seg_src = start_index.reshape(R, E).T.reshape(-1)
    pos = np.arange(max_len, dtype=np.int64)
    idx = np.clip(np.searchsorted(seg_ws, pos, side="right") - 1, 0, E * R - 1)
    within = pos - seg_ws[idx]
    valid = (within >= 0) & (within < seg_len[idx])
    perm = np.where(valid, seg_src[idx] + within, T)
    return perm.astype(np.int64), m_sizes, (m_offsets - m_sizes)


def _partitions(c, max_part, max_len):
    """Partitions of c into <= max_len parts each <= max_part, desc order."""
    out = []

    def rec(rem, mx, cur):
        if rem == 0:
            out.append(tuple(cur))
            return
        if len(cur) == max_len:
            return
        for p in range(min(rem, mx), 0, -1):
            cur.append(p)
            rec(rem - p, p, cur)
            cur.pop()

    rec(c, max_part, [])
    return out


def _ffd(nblk, parts):
    """Pack per-expert block counts into 8 cores x len(parts) bins
    (bin (c,j) capacity parts[j], single expert per bin). Returns
    assignment dict (core, j) -> (expert, block_start, nb) or None."""
    bins = []  # (cap, core, j)
    for c in range(NCORES):
        for j, cap in enumerate(parts):
            bins.append([cap, c, j])
    bins.sort(key=lambda b: -b[0])
    used = [False] * len(bins)
    asg = {}
    order = sorted(range(E), key=lambda e: -nblk[e])
    for e in order:
        rem = int(nblk[e])
        b0 = 0
        while rem > 0:
            # largest unused bin with cap <= rem (fill fully); else the
            # smallest unused bin (minimize slack)
            best_le, best_gt = None, None
            for i, (cap, c, j) in enumerate(bins):
                if used[i]:
                    continue
                if cap <= rem:
                    best_le = i  # bins desc: first such is largest
                    break
                best_gt = i  # keeps updating: last seen = smallest so far
            i = best_le if best_le is not None else best_gt
            if i is None:
                return None
            cap, c, j = bins[i]
            used[i] = True
            nb = min(cap, rem)
            asg[(c, j)] = (e, b0, nb)
            b0 += nb
            rem -= nb
    return asg


def _plan(nblk):
    """nblk: per-expert sizes in UNIT-row units. Returns (parts, asg) with
    parts = per-segment capacities in units (same on all cores)."""
    n = int(sum(nblk))
    c0 = max(1, -(-n // NCORES))
    # a segment costs a full weight stream regardless of size, so prefer
    # <=2 segments even at a few extra rows before allowing 3
    for max_len, c_hi in ((2, c0 + 8), (3, c0 + 8), (3, 600)):
        for c in range(c0, c_hi):
            opts = _partitions(c, MAX_UNITS, max_len)
            # most balanced first (largest min part)
            for parts in sorted(opts, key=lambda p: (len(p), -min(p))):
                asg = _ffd(nblk, parts)
                if asg is not None:
                    return list(parts), asg
    raise RuntimeError("packing failed")


def _seg_kinds(parts):
    return ["small" if U * UNIT <= SMALL_ROWS else "big" for U in parts]


def _build_program(parts):
    import concourse.mybir as mybir
    import concourse.tile as tile
    from concourse import bacc

    bf = mybir.dt.bfloat16
    f8 = mybir.dt.float8e3
    f32 = mybir.dt.float32
    SILU = mybir.ActivationFunctionType.Silu

    nc = bacc.Bacc("TRN2", target_bir_lowering=False, debug=False,
                   num_devices=NCORES)

    kinds = _seg_kinds(parts)
    ident_d = None
    if "small" in kinds:
        ident_d = nc.dram_tensor("ident", [P, P], bf, kind="ExternalInput")
    dram = []
    for j, U in enumerate(parts):
        M = U * UNIT
        t = {}
        t["xt"] = nc.dram_tensor(f"xt{j}", [P, DT * M], bf, kind="ExternalInput")
        if kinds[j] == "big":
            t["w1"] = nc.dram_tensor(f"w1p{j}", [HT, P, D], bf, kind="ExternalInput")
            t["w3"] = nc.dram_tensor(f"w3p{j}", [HT, P, D], bf, kind="ExternalInput")
            t["w2"] = nc.dram_tensor(f"w2p{j}", [H, D], bf, kind="ExternalInput")
            t["out"] = nc.dram_tensor(f"out{j}", [P, DT * M], bf, kind="ExternalOutput")
        else:
            t["w1"] = nc.dram_tensor(f"w1p{j}", [P, DT * H], f8, kind="ExternalInput")
            t["w3"] = nc.dram_tensor(f"w3p{j}", [P, DT * H], f8, kind="ExternalInput")
            t["w2"] = nc.dram_tensor(f"w2p{j}", [P, HT * D], f8, kind="ExternalInput")
            t["out"] = nc.dram_tensor(f"out{j}", [M, D], bf, kind="ExternalOutput")
        dram.append(t)

    with tile.TileContext(nc) as tc:
        with (
            tc.tile_pool(name="xtb", bufs=4) as xtb_pool,    # big token chunks
            tc.tile_pool(name="xts", bufs=2) as xts_pool,    # small token tile
            tc.tile_pool(name="w8", bufs=5) as w8_pool,      # w1/w3 streams (8KB slots)
            tc.tile_pool(name="w2", bufs=10) as w2_pool,     # w2 streams (8KB slots)
            tc.tile_pool(name="ht", bufs=8) as ht_pool,      # big h tiles
            tc.tile_pool(name="htp", bufs=1) as htp_pool,    # small h' [M, H]
            tc.tile_pool(name="htt", bufs=1) as htt_pool,    # small h transposed
            tc.tile_pool(name="tmp", bufs=2) as tmp_pool,    # silu temp
            tc.tile_pool(name="ob", bufs=2) as ob_pool,      # output staging
            tc.tile_pool(name="warm", bufs=1) as warm_pool,
            tc.tile_pool(name="idp", bufs=1) as id_pool,
            tc.tile_pool(name="pacc", bufs=4, space="PSUM") as pacc_pool,
            tc.tile_pool(name="pout", bufs=3, space="PSUM") as pout_pool,
            tc.tile_pool(name="pwarm", bufs=1, space="PSUM") as pwarm_pool,
        ):
            # keep the PE busy while the first weight/token DMAs land
            wz = warm_pool.tile([P, P], bf, tag="warm", name="warmz")
            nc.vector.memset(wz[:], 0.0)
            pw = pwarm_pool.tile([P, P], f32, tag="pwarm", name="warmp")
            for _ in range(20):
                nc.tensor.matmul(pw[:], wz[:], wz[:], start=True, stop=True)
            ident = None
            if ident_d is not None:
                ident = id_pool.tile([P, P], bf, tag="id", name="identt")
                nc.sync.dma_start(out=ident[:], in_=ident_d[:, :])

            for j, U in enumerate(parts):
                M = U * UNIT
                t = dram[j]
                if kinds[j] == "big":
                    # ---- tokens: 4 chunks of 8 d-tiles, SWDGE queue ----
                    xch = []
                    for c in range(4):
                        xt = xtb_pool.tile([P, 8 * M], bf, tag="xtb",
                                           name=f"xt{j}_{c}")
                        nc.scalar.dma_start(
                            out=xt[:], in_=t["xt"][:, c * 8 * M:(c + 1) * 8 * M])
                        xch.append(xt)

                    def load_wp(h, j=j, t=t):
                        w1t = w8_pool.tile([P, D], bf, tag="w8", name=f"w1_{j}_{h}")
                        nc.sync.dma_start(out=w1t[:], in_=t["w1"][h])
                        w3t = w8_pool.tile([P, D], bf, tag="w8", name=f"w3_{j}_{h}")
                        nc.sync.dma_start(out=w3t[:], in_=t["w3"][h])
                        return w1t, w3t

                    wp_cur = load_wp(0)
                    hts, w2s = [], []
                    for h in range(HT):
                        w1t, w3t = wp_cur
                        if h + 1 < HT:
                            wp_cur = load_wp(h + 1)
                        # w2 panels stream through phase 1 on the sync queue
                        w2t = w2_pool.tile([P, D], bf, tag="w2", name=f"w2_{j}_{h}")
                        nc.sync.dma_start(out=w2t[:], in_=t["w2"][h * P:(h + 1) * P, :])
                        w2s.append(w2t)
                        ps1 = pacc_pool.tile([P, M], f32, tag="pacc", name=f"ps1_{j}_{h}")
                        ps3 = pacc_pool.tile([P, M], f32, tag="pacc", name=f"ps3_{j}_{h}")
                        for d in range(DT):
                            rhs = xch[d // 8][:, (d % 8) * M:(d % 8 + 1) * M]
                            nc.tensor.matmul(ps1[:], w1t[:, d * P:(d + 1) * P], rhs,
                                             start=(d == 0), stop=(d == DT - 1))
                            nc.tensor.matmul(ps3[:], w3t[:, d * P:(d + 1) * P], rhs,
                                             start=(d == 0), stop=(d == DT - 1))
                        tmp = tmp_pool.tile([P, max(M, 512)], f32, tag="tmp",
                                            name=f"tmp_{j}_{h}")
                        nc.scalar.activation(tmp[:, :M], ps1[:], SILU)
                        ht = ht_pool.tile([P, M], bf, tag="ht", name=f"ht_{j}_{h}")
                        nc.vector.tensor_mul(ht[:], tmp[:, :M], ps3[:])
                        hts.append(ht)
                    # ---- phase 2 (w2 stationary): outT[dt] = sum_h w2[h,dt].T @ ht[h]
                    ob = None
                    for dt in range(DT):
                        po = pout_pool.tile([P, max(M, 512)], f32, tag="pout",
                                            name=f"po_{j}_{dt}")
                        for h in range(HT):
                            nc.tensor.matmul(po[:, :M], w2s[h][:, dt * P:(dt + 1) * P],
                                             hts[h][:], start=(h == 0),
                                             stop=(h == HT - 1))
                        if dt % 8 == 0:
                            ob = ob_pool.tile([P, max(8 * M, D)], bf, tag="ob",
                                              name=f"ob_{j}_{dt // 8}")
                        nc.vector.tensor_copy(ob[:, (dt % 8) * M:(dt % 8 + 1) * M],
                                              po[:, :M])
                        if dt % 8 == 7:
                            c = dt // 8
                            nc.scalar.dma_start(
                                out=t["out"][:, c * 8 * M:(c + 1) * 8 * M],
                                in_=ob[:, :8 * M])
                else:
                    # ---- small segment: tokens stationary, fp8 weights moving
                    xt = xts_pool.tile([P, DT * M], bf, tag="xts", name=f"xts{j}")
                    nc.scalar.dma_start(out=xt[:], in_=t["xt"][:])
                    w13 = []
                    for c in range(4):
                        w1t = w8_pool.tile([P, 8 * H], f8, tag="w8", name=f"w1s_{j}_{c}")
                        nc.sync.dma_start(out=w1t[:],
                                          in_=t["w1"][:, c * 8 * H:(c + 1) * 8 * H])
                        w3t = w8_pool.tile([P, 8 * H], f8, tag="w8", name=f"w3s_{j}_{c}")
                        nc.sync.dma_start(out=w3t[:],
                                          in_=t["w3"][:, c * 8 * H:(c + 1) * 8 * H])
                        w13.append((w1t, w3t))
                    w2c = []
                    for c in range(4):
                        w2t = w2_pool.tile([P, 2 * D], f8, tag="w2", name=f"w2s_{j}_{c}")
                        nc.sync.dma_start(out=w2t[:],
                                          in_=t["w2"][:, c * 2 * D:(c + 1) * 2 * D])
                        w2c.append(w2t)
                    pa = [pacc_pool.tile([P, 512], f32, tag="pacc", name=f"pa_{j}_{i}")
                          for i in range(2)]
                    pb = [pacc_pool.tile([P, 512], f32, tag="pacc", name=f"pb_{j}_{i}")
                          for i in range(2)]
                    for d in range(DT):
                        lhsT = xt[:, d * M:(d + 1) * M]
                        w1t, w3t = w13[d // 8]
                        off = (d % 8) * H
                        st, sp = (d == 0), (d == DT - 1)
                        nc.tensor.matmul(pa[0][:M, :], lhsT, w1t[:, off:off + 512],
                                         start=st, stop=sp)
                        nc.tensor.matmul(pa[1][:M, :], lhsT, w1t[:, off + 512:off + 1024],
                                         start=st, stop=sp)
                        nc.tensor.matmul(pb[0][:M, :], lhsT, w3t[:, off:off + 512],
                                         start=st, stop=sp)
                        nc.tensor.matmul(pb[1][:M, :], lhsT, w3t[:, off + 512:off + 1024],
                                         start=st, stop=sp)
                    htp = htp_pool.tile([P, H], bf, tag="htp", name=f"htp{j}")
                    for i in range(2):
                        tmp = tmp_pool.tile([P, max(512, 512)], f32, tag="tmp",
                                            name=f"tmps_{j}_{i}")
                        nc.scalar.activation(tmp[:M, :512], pa[i][:M, :], SILU,
                                             scale=1.0 / S1)
                        nc.vector.tensor_mul(htp[:M, i * 512:(i + 1) * 512],
                                             tmp[:M, :512], pb[i][:M, :])
                    # h' [M, H] -> hT [h, M] via PE transpose (keeps HAM warm)
                    htt = htt_pool.tile([P, HT * M], bf, tag="htt", name=f"htt{j}")
                    for h in range(HT):
                        pt = pout_pool.tile([P, max(M, 512)], bf, tag="pout",
                                            name=f"pt_{j}_{h}")
                        nc.tensor.transpose(pt[:, :M], htp[:M, h * P:(h + 1) * P],
                                            ident[:])
                        nc.vector.tensor_copy(htt[:, h * M:(h + 1) * M], pt[:, :M])
                    obs = ob_pool.tile([P, max(8 * M, D)], bf, tag="ob", name=f"obs{j}")
                    for dc in range(D // 512):
                        po = pout_pool.tile([P, 512], f32, tag="pout", name=f"pos_{j}_{dc}")
                        for h in range(HT):
                            w2t = w2c[h // 2]
                            off = (h % 2) * D + dc * 512
                            nc.tensor.matmul(po[:M, :], htt[:, h * M:(h + 1) * M],
                                             w2t[:, off:off + 512],
                                             start=(h == 0), stop=(h == HT - 1))
                        nc.vector.tensor_scalar_mul(obs[:M, dc * 512:(dc + 1) * 512],
                                                    po[:M, :], 1.0 / S2)
                        if dc == 3:
                            nc.scalar.dma_start(out=t["out"][:, :2048],
                                                in_=obs[:M, :2048])
                    nc.scalar.dma_start(out=t["out"][:, 2048:], in_=obs[:M, 2048:D])

    nc.compile()
    return nc


_CACHE = {}


def _get_program(parts):
    key = tuple(parts)
    if key not in _CACHE:
        _CACHE[key] = _build_program(parts)
    return _CACHE[key]


_LAST_RESULT = None


def kernel(x, w1, w2, w3, num_tokens_per_expert):
    import os
    from concourse.bass_utils import run_bass_kernel_spmd

    x = np.asarray(x, dtype=np.float32)
    w1 = np.asarray(w1, np.float32)
    w2 = np.asarray(w2, np.float32)
    w3 = np.asarray(w3, np.float32)
    counts = np.asarray(num_tokens_per_expert).astype(np.int64)
    perm, m_sizes, m_off = _permute_indices(counts)
    nunits = m_sizes // UNIT  # m_sizes are UNIT-aligned

    parts, asg = _plan(nunits)
    kinds = _seg_kinds(parts)
    nc = _get_program(parts)

    # expert-grouped token stream (the dispatch): rows of x per expert
    x_pad = np.concatenate([x, np.zeros((1, D), np.float32)], axis=0)
    ltot = int(m_sizes.sum())
    xp = x_pad[perm[:ltot]]  # [ltot, D] expert-grouped, 16-aligned per expert
    xe = [xp[m_off[e]:m_off[e] + m_sizes[e]] for e in range(E)]

    big_cache, small_cache = {}, {}

    def big_w(e):
        if e not in big_cache:
            w1b = np.ascontiguousarray(
                w1[e].reshape(DT, P, HT, P).transpose(2, 1, 0, 3)
                .reshape(HT, P, D)).astype(BF16)
            w3b = np.ascontiguousarray(
                w3[e].reshape(DT, P, HT, P).transpose(2, 1, 0, 3)
                .reshape(HT, P, D)).astype(BF16)
            w2b = w2[e].astype(BF16)
            big_cache[e] = (w1b, w3b, w2b)
        return big_cache[e]

    def small_w(e):
        if e not in small_cache:
            q1 = np.ascontiguousarray(
                np.clip(w1[e] * S1, -15.0, 15.0).reshape(DT, P, H)
                .transpose(1, 0, 2).reshape(P, DT * H)).astype(F8E3)
            q3 = np.ascontiguousarray(
                np.clip(w3[e] * S1, -15.0, 15.0).reshape(DT, P, H)
                .transpose(1, 0, 2).reshape(P, DT * H)).astype(F8E3)
            q2 = np.ascontiguousarray(
                np.clip(w2[e] * (S2 / S1), -15.0, 15.0).reshape(HT, P, D)
                .transpose(1, 0, 2).reshape(P, HT * D)).astype(F8E3)
            small_cache[e] = (q1, q3, q2)
        return small_cache[e]

    ident_np = np.eye(P, dtype=np.float32).astype(BF16)
    in_maps = []
    for c in range(NCORES):
        m = {}
        if "small" in kinds:
            m["ident"] = ident_np
        for j, U in enumerate(parts):
            M = U * UNIT
            ent = asg.get((c, j))
            e = ent[0] if ent is not None else 0
            blk = np.zeros((M, D), np.float32)
            if ent is not None:
                _, u0, nu = ent
                blk[:nu * UNIT] = xe[e][u0 * UNIT:(u0 + nu) * UNIT]
            # [P, DT*M]: xt[p, d*M + m] = blk[m, d*P + p]
            m[f"xt{j}"] = np.ascontiguousarray(
                blk.reshape(M, DT, P).transpose(2, 1, 0).reshape(P, DT * M)
            ).astype(BF16)
            w1b, w3b, w2b = big_w(e) if kinds[j] == "big" else small_w(e)
            m[f"w1p{j}"] = w1b
            m[f"w3p{j}"] = w3b
            m[f"w2p{j}"] = w2b
        in_maps.append(m)

    kw = {}
    if os.environ.get("KERNEL_TRACE"):
        kw = dict(trace=True, tmpdir=os.environ.get("KERNEL_TRACE_DIR") or None)
    res = run_bass_kernel_spmd(nc, in_maps, core_ids=list(range(NCORES)), **kw)
    global _LAST_RESULT
    _LAST_RESULT = res

    # reassemble expert-grouped output stream, then scatter to token order
    outp = np.zeros((ltot, D), np.float32)
    for (c, j), (e, u0, nu) in asg.items():
        nr = nu * UNIT
        M = parts[j] * UNIT
        r = np.asarray(res.results[c][f"out{j}"], dtype=np.float32)
        if kinds[j] == "big":
            # [P, DT*M] -> [M, D]
            r = r.reshape(P, DT, M).transpose(2, 1, 0).reshape(M, D)
        outp[m_off[e] + u0 * UNIT:m_off[e] + u0 * UNIT + nr] = r[:nr]

    out = np.zeros((T + 1, D), np.float32)
    out[perm[:ltot]] = outp
    return out[:T]
